# revision 1
# baseline (speedup 1.0000x reference)
"""Trainium2 Bass kernel for seq2seq LSTM encoder/decoder cross-entropy loss.

Strategy (8 NeuronCores, SPMD):
  - LSTM encoder (50 steps) + decoder (49 steps) replicated on all cores.
  - Output projection (512 -> 32000 vocab) tensor-parallel: each core owns a
    4000-row vocab shard of W_out/b_out, kept SBUF-resident, and computes
    per-(step,batch) partial sum-of-exp(logits) plus the target-token logit
    for targets its shard owns.
  - Host combines: loss = sum_t mean_b( ln(sum_c s_c) - sum_c tau_c ).

Phases on device (per core):
  P0  load weights to SBUF; gather token embeddings (indirect DMA) and
      transpose them hidden-major via the PE.
  P1  precompute X = x @ W_ih^T + (b_ih+b_hh) for all steps -> DRAM scratch.
  P2  encoder recurrence (sigmoid/tanh activation table).
  P3  decoder recurrence; saves transposed hidden states; computes target
      logit tau per step via gathered W_out rows (DVE dot with fused reduce).
  P4  projection: logits tiles -> exp (with fused accumulate) -> sum-of-exp.
"""

import functools
import numpy as np
import ml_dtypes

import concourse.bacc as bacc
import concourse.bass as bass
import concourse.mybir as mybir
from concourse import tile

BF16 = mybir.dt.bfloat16
F32 = mybir.dt.float32
I32 = mybir.dt.int32

H = 512
KC = 4  # hidden chunks of 128
B = 64
G = 4 * H  # 2048 gates
VOCAB = 32000
NCORES = 8
VSH = VOCAB // NCORES  # 4000 unpadded shard
AF = mybir.ActivationFunctionType


def _cfg(enc_steps=50, dec_steps=49, v_ntiles=8):
    assert enc_steps % 2 == 0
    mt = (dec_steps * B + 127) // 128  # decoder M-tiles of 128 rows
    return dict(
        enc_steps=enc_steps,
        dec_steps=dec_steps,
        v_ntiles=v_ntiles,  # vocab N-tiles of 512 per core
        vs_pad=v_ntiles * 512,  # padded vocab shard size
        emb_rows=VOCAB,
        enc_mt=enc_steps * B // 128,
        dec_mt=mt,
    )


def build_program(cfg):
    """Emit the full Bass/Tile program. Returns the compiled Bacc."""
    enc_steps, dec_steps = cfg["enc_steps"], cfg["dec_steps"]
    enc_mt, dec_mt = cfg["enc_mt"], cfg["dec_mt"]
    vnt, vsp = cfg["v_ntiles"], cfg["vs_pad"]
    emb_rows = cfg["emb_rows"]

    nc = bacc.Bacc(
        "TRN2", target_bir_lowering=False, debug=False, num_devices=NCORES
    )

    def din(name, shape, dt):
        return nc.dram_tensor(name, list(shape), dt, kind="ExternalInput").ap()

    def dout(name, shape, dt):
        return nc.dram_tensor(name, list(shape), dt, kind="ExternalOutput").ap()

    emb_in = din("emb_in", (emb_rows, H), BF16)
    emb_tgt = din("emb_tgt", (emb_rows, H), BF16)
    wih_enc = din("wih_enc", (H, G), BF16)  # W_ih^T, gate-reordered
    whh_enc = din("whh_enc", (H, G), BF16)
    wih_dec = din("wih_dec", (H, G), BF16)
    whh_dec = din("whh_dec", (H, G), BF16)
    bias_enc = din("bias_enc", (128, G), F32)  # (b_ih+b_hh) broadcast
    bias_dec = din("bias_dec", (128, G), F32)
    wout_t = din("wout_t", (H, vsp), BF16)  # W_out^T shard, padded
    b_row = din("b_row", (1, vsp), BF16)  # b_out shard row (pad -100)
    waug = din("waug", (vsp, 516), F32)  # [W_sh | b_sh | 0 0 0], row vsp-? pad
    ident = din("ident", (128, 128), BF16)
    ones_col = din("ones_col", (1, 128), BF16)
    etok = din("etok", (128, enc_mt), I32)
    dtok = din("dtok", (128, dec_mt), I32)
    ttok = din("ttok", (B, dec_steps), I32)

    s_out = dout("s_out", (128, dec_mt), F32)
    t_out = dout("t_out", (B, dec_steps), F32)

    xih_enc = nc.dram_tensor("xih_enc", [enc_mt * 128, G], BF16, kind="Internal").ap()
    xih_dec = nc.dram_tensor("xih_dec", [dec_mt * 128, G], BF16, kind="Internal").ap()

    with tile.TileContext(nc) as tc:
        with tc.tile_pool(name="persist", bufs=1) as pp:
            # --- persistent SBUF residents ---
            whh_e = pp.tile([128, KC * G], BF16)  # 4 chunks side by side
            whh_d = pp.tile([128, KC * G], BF16)
            wout_s = pp.tile([128, KC * vsp], BF16)
            bias_e = pp.tile([128, G], F32)
            bias_d = pp.tile([128, G], F32)
            id_sb = pp.tile([128, 128], BF16)
            ones_sb = pp.tile([1, 128], BF16)
            brow_sb = pp.tile([1, vsp], BF16)
            etok_sb = pp.tile([128, enc_mt], I32)
            dtok_sb = pp.tile([128, dec_mt], I32)
            ttok_sb = pp.tile([B, dec_steps], I32)
            hT_all = pp.tile([128, KC * dec_mt * 128], BF16)  # dec hT chunks
            s_all = pp.tile([128, dec_mt], F32)
            t_all = pp.tile([B, dec_steps], F32)

            for k in range(KC):
                nc.sync.dma_start(whh_e[:, k * G:(k + 1) * G], whh_enc[k * 128:(k + 1) * 128, :])
                nc.sync.dma_start(whh_d[:, k * G:(k + 1) * G], whh_dec[k * 128:(k + 1) * 128, :])
                nc.sync.dma_start(wout_s[:, k * vsp:(k + 1) * vsp], wout_t[k * 128:(k + 1) * 128, :])
            nc.sync.dma_start(bias_e[:], bias_enc[:])
            nc.sync.dma_start(bias_d[:], bias_dec[:])
            nc.sync.dma_start(id_sb[:], ident[:])
            nc.sync.dma_start(ones_sb[:], ones_col[:])
            nc.sync.dma_start(brow_sb[:], b_row[:])
            nc.sync.dma_start(etok_sb[:], etok[:])
            nc.sync.dma_start(dtok_sb[:], dtok[:])
            nc.sync.dma_start(ttok_sb[:], ttok[:])

            whh_e_v = whh_e[:].rearrange("p (k g) -> p k g", k=KC)
            whh_d_v = whh_d[:].rearrange("p (k g) -> p k g", k=KC)
            wout_v = wout_s[:].rearrange("p (k v) -> p k v", k=KC)
            hT_all_v = hT_all[:].rearrange("p (k t) -> p k t", k=KC)

            # =========== P0+P1: gather + transpose + X_ih precompute ===========
            with tc.tile_pool(name="pre", bufs=1) as pr, \
                 tc.tile_pool(name="pre_ps", bufs=1, space="PSUM") as prp:
                wih_e = pr.tile([128, KC * G], BF16)
                wih_d = pr.tile([128, KC * G], BF16)
                for k in range(KC):
                    nc.sync.dma_start(wih_e[:, k * G:(k + 1) * G], wih_enc[k * 128:(k + 1) * 128, :])
                    nc.sync.dma_start(wih_d[:, k * G:(k + 1) * G], wih_dec[k * 128:(k + 1) * 128, :])
                wih_e_v = wih_e[:].rearrange("p (k g) -> p k g", k=KC)
                wih_d_v = wih_d[:].rearrange("p (k g) -> p k g", k=KC)

                for src in range(2):
                    n_mt = enc_mt if src == 0 else dec_mt
                    tok_sb = etok_sb if src == 0 else dtok_sb
                    emb = emb_in if src == 0 else emb_tgt
                    wih_v = wih_e_v if src == 0 else wih_d_v
                    bias_sb = bias_e if src == 0 else bias_d
                    xih = xih_enc if src == 0 else xih_dec
                    for mt in range(n_mt):
                        xr = pr.tile([128, H], BF16, tag="xrows", bufs=3)
                        nc.gpsimd.indirect_dma_start(
                            out=xr[:], out_offset=None, in_=emb[:],
                            in_offset=bass.IndirectOffsetOnAxis(
                                ap=tok_sb[:, mt:mt + 1], axis=0),
                        )
                        tps = prp.tile([128, H], BF16, tag="tps", bufs=2)
                        for k in range(KC):
                            nc.tensor.transpose(
                                out=tps[:, k * 128:(k + 1) * 128],
                                in_=xr[:, k * 128:(k + 1) * 128],
                                identity=id_sb[:],
                            )
                        xT = pr.tile([128, H], BF16, tag="xT", bufs=3)
                        nc.vector.tensor_copy(out=xT[:], in_=tps[:])
                        gps = [prp.tile([128, 512], F32, tag="gp", bufs=6,
                                        name=f"gp{mt}_{n}")
                               for n in range(4)]
                        for k in range(KC):
                            for n in range(4):
                                nc.tensor.matmul(
                                    out=gps[n][:],
                                    lhsT=xT[:, k * 128:(k + 1) * 128],
                                    rhs=wih_v[:, k, n * 512:(n + 1) * 512],
                                    start=(k == 0), stop=(k == KC - 1),
                                )
                        xg = pr.tile([128, G], BF16, tag="xg", bufs=3)
                        for n in range(4):
                            nc.vector.tensor_add(
                                out=xg[:, n * 512:(n + 1) * 512], in0=gps[n][:],
                                in1=bias_sb[:, n * 512:(n + 1) * 512])
                        nc.sync.dma_start(xih[mt * 128:(mt + 1) * 128, :], xg[:])

            # =========== P2+P3: recurrences ===========
            with tc.tile_pool(name="rec", bufs=1) as rc, \
                 tc.tile_pool(name="rec_ps", bufs=1, space="PSUM") as rcp:

                hT_prev = rc.tile([128, KC * B], BF16, tag="hT0")
                nc.gpsimd.memset(hT_prev[:], 0.0)
                c_prev = rc.tile([B, H], F32, tag="c0")
                nc.gpsimd.memset(c_prev[:], 0.0)
                hT_prev_v = hT_prev[:].rearrange("p (k b) -> p k b", k=KC)

                def lstm_step(t, is_dec):
                    nonlocal hT_prev_v, c_prev
                    whh_v = whh_d_v if is_dec else whh_e_v
                    xih = xih_dec if is_dec else xih_enc
                    bias_sb = bias_d if is_dec else bias_e  # noqa (bias already in xih)

                    xt = rc.tile([B, G], BF16, tag="xt", bufs=3)
                    nc.sync.dma_start(xt[:], xih[t * B:(t + 1) * B, :])

                    gp = rcp.tile([B, G], F32, tag="gates", bufs=1)
                    for k in range(KC):
                        for n in range(4):
                            nc.tensor.matmul(
                                out=gp[:, n * 512:(n + 1) * 512],
                                lhsT=hT_prev_v[:, k, :],
                                rhs=whh_v[:, k, n * 512:(n + 1) * 512],
                                start=(k == 0), stop=(k == KC - 1),
                            )
                    gsb = rc.tile([B, G], F32, tag="gsb", bufs=2)
                    nc.vector.tensor_add(out=gsb[:], in0=gp[:], in1=xt[:])

                    # gate order (host-permuted): f, i, o, g
                    sig = rc.tile([B, 3 * H], F32, tag="sig", bufs=2)
                    nc.scalar.activation(out=sig[:], in_=gsb[:, 0:3 * H], func=AF.Sigmoid)
                    tg = rc.tile([B, H], F32, tag="tg", bufs=2)
                    nc.scalar.activation(out=tg[:], in_=gsb[:, 3 * H:4 * H], func=AF.Tanh)

                    t1 = rc.tile([B, H], F32, tag="t1", bufs=2)
                    nc.vector.tensor_mul(out=t1[:], in0=sig[:, H:2 * H], in1=tg[:])
                    t2 = rc.tile([B, H], F32, tag="t2", bufs=2)
                    nc.vector.tensor_mul(out=t2[:], in0=sig[:, 0:H], in1=c_prev[:])
                    c_new = rc.tile([B, H], F32, tag="c", bufs=2)
                    nc.vector.tensor_add(out=c_new[:], in0=t1[:], in1=t2[:])
                    tc_ = rc.tile([B, H], F32, tag="tc", bufs=2)
                    nc.scalar.activation(out=tc_[:], in_=c_new[:], func=AF.Tanh)
                    h = rc.tile([B, H], BF16, tag="h", bufs=3)
                    nc.vector.tensor_mul(out=h[:], in0=sig[:, 2 * H:3 * H], in1=tc_[:])

                    # transpose h -> hT chunks [128, 64] each
                    tps = rcp.tile([128, KC * B], BF16, tag="tpsh", bufs=2)
                    for k in range(KC):
                        nc.tensor.transpose(
                            out=tps[:, k * B:(k + 1) * B],
                            in_=h[:, k * 128:(k + 1) * 128],
                            identity=id_sb[0:B, 0:B],
                        )
                    if is_dec:
                        hT_dst = hT_all_v[:, :, t * B:(t + 1) * B]
                        nc.vector.tensor_copy(out=hT_dst, in_=tps[:])
                        hT_prev_v = hT_all_v[:, :, t * B:(t + 1) * B]
                        # tau: gather W_out rows of target tokens, fused dot
                        wt = rc.tile([B, 516], F32, tag="wt", bufs=3)
                        nc.gpsimd.indirect_dma_start(
                            out=wt[:], out_offset=None, in_=waug[:],
                            in_offset=bass.IndirectOffsetOnAxis(
                                ap=ttok_sb[:, t:t + 1], axis=0),
                        )
                        prod = rc.tile([B, H], F32, tag="prod", bufs=2)
                        nc.vector.tensor_mul(out=prod[:], in0=h[:], in1=wt[:, 0:H])
                        tau0 = rc.tile([B, 1], F32, tag="tau0", bufs=2)
                        nc.vector.tensor_reduce(
                            out=tau0[:], in_=prod[:],
                            axis=mybir.AxisListType.X, op=mybir.AluOpType.add)
                        nc.vector.tensor_add(
                            out=t_all[:, t:t + 1], in0=tau0[:], in1=wt[:, 512:513])
                    else:
                        hT_new = rc.tile([128, KC * B], BF16, tag="hTs", bufs=2)
                        nc.vector.tensor_copy(out=hT_new[:], in_=tps[:])
                        hT_prev_v = hT_new[:].rearrange("p (k b) -> p k b", k=KC)
                    c_prev = c_new

                for t in range(enc_steps):
                    lstm_step(t, False)
                for t in range(dec_steps):
                    lstm_step(t, True)

                # zero pad region of hT_all (columns beyond last dec step)
                if dec_steps * B < dec_mt * 128:
                    nc.gpsimd.memset(hT_all_v[:, :, dec_steps * B:dec_mt * 128], 0.0)

            # =========== P4: projection + softmax partials ===========
            with tc.tile_pool(name="proj", bufs=1) as pj, \
                 tc.tile_pool(name="proj_ps", bufs=1, space="PSUM") as pjp:
                for mt in range(dec_mt):
                    sparts = pj.tile([128, vnt], F32, tag="sparts", bufs=2)
                    for n in range(vnt):
                        lp = pjp.tile([128, 512], F32, tag="lp", bufs=4)
                        nc.tensor.matmul(
                            out=lp[:], lhsT=ones_sb[:],
                            rhs=brow_sb[:, n * 512:(n + 1) * 512],
                            start=True, stop=False,
                        )
                        for k in range(KC):
                            nc.tensor.matmul(
                                out=lp[:],
                                lhsT=hT_all_v[:, k, mt * 128:(mt + 1) * 128],
                                rhs=wout_v[:, k, n * 512:(n + 1) * 512],
                                start=False, stop=(k == KC - 1),
                            )
                        ex = pj.tile([128, 512], F32, tag="ex", bufs=3)
                        nc.scalar.activation(
                            out=ex[:], in_=lp[:], func=AF.Exp,
                            accum_out=sparts[:, n:n + 1],
                        )
                    nc.vector.tensor_reduce(
                        out=s_all[:, mt:mt + 1], in_=sparts[:],
                        axis=mybir.AxisListType.X, op=mybir.AluOpType.add,
                    )

            nc.sync.dma_start(s_out[:], s_all[:])
            nc.sync.dma_start(t_out[:], t_all[:])

    nc.compile()
    return nc


# ============================ host side ============================

_GATE_PERM = np.r_[512:1024, 0:512, 1536:2048, 1024:1536]  # i,f,g,o -> f,i,o,g


def _prep_shared(inputs, cfg):
    """Host-side packing of replicated (non-sharded) device inputs."""
    bf = ml_dtypes.bfloat16
    enc_steps, dec_steps = cfg["enc_steps"], cfg["dec_steps"]
    enc_mt, dec_mt = cfg["enc_mt"], cfg["dec_mt"]

    def wT(w):
        return np.ascontiguousarray(w[_GATE_PERM].T).astype(bf)

    def biasb(bi, bh):
        b = (np.asarray(bi, np.float32) + np.asarray(bh, np.float32))[_GATE_PERM]
        return np.ascontiguousarray(np.broadcast_to(b, (128, G))).astype(np.float32)

    il = np.asarray(inputs["input_lines"]).astype(np.int64)[:enc_steps]
    tl = np.asarray(inputs["target_lines"]).astype(np.int64)[: dec_steps + 1]
    etok_flat = il.reshape(-1)  # step-major (t*64+b)
    dtok_flat = tl[:-1].reshape(-1)
    dtok_flat = np.concatenate([
        dtok_flat, np.zeros(dec_mt * 128 - dtok_flat.size, np.int64)])
    tgt_next = tl[1:].reshape(-1)  # [dec_steps*64]

    shared = {
        "emb_in": np.asarray(inputs["emb_in"], np.float32).astype(bf),
        "emb_tgt": np.asarray(inputs["emb_tgt"], np.float32).astype(bf),
        "wih_enc": wT(np.asarray(inputs["W_ih_enc"], np.float32)),
        "whh_enc": wT(np.asarray(inputs["W_hh_enc"], np.float32)),
        "wih_dec": wT(np.asarray(inputs["W_ih_dec"], np.float32)),
        "whh_dec": wT(np.asarray(inputs["W_hh_dec"], np.float32)),
        "bias_enc": biasb(inputs["b_ih_enc"], inputs["b_hh_enc"]),
        "bias_dec": biasb(inputs["b_ih_dec"], inputs["b_hh_dec"]),
        "ident": np.eye(128, dtype=bf),
        "ones_col": np.ones((1, 128), dtype=bf),
        "etok": np.ascontiguousarray(
            etok_flat.reshape(enc_mt, 128).T).astype(np.int32),
        "dtok": np.ascontiguousarray(
            dtok_flat.reshape(dec_mt, 128).T).astype(np.int32),
    }
    return shared, tgt_next


def _prep_core(inputs, cfg, core, tgt_next):
    """Per-core vocab-shard inputs."""
    bf = ml_dtypes.bfloat16
    vnt, vsp = cfg["v_ntiles"], cfg["vs_pad"]
    dec_steps = cfg["dec_steps"]
    W_out = np.asarray(inputs["W_out"], np.float32)
    b_out = np.asarray(inputs["b_out"], np.float32)
    vocab = W_out.shape[0]
    vs = vocab // NCORES
    lo, hi = core * vs, (core + 1) * vs
    w_sh = W_out[lo:hi]  # [vs, H]
    b_sh = b_out[lo:hi]

    wout_t = np.zeros((H, vsp), np.float32)
    wout_t[:, :vs] = w_sh.T
    b_row = np.full((1, vsp), -100.0, np.float32)
    b_row[0, :vs] = b_sh

    waug = np.zeros((vsp, 516), np.float32)
    waug[:vs, :H] = w_sh
    waug[:vs, 512] = b_sh

    tloc = tgt_next - lo
    tloc = np.where((tloc >= 0) & (tloc < vs), tloc, vs).astype(np.int32)
    # route out-of-shard targets to a zero row (vs < vsp always)
    ttok = np.ascontiguousarray(tloc.reshape(dec_steps, B).T).astype(np.int32)

    return {
        "wout_t": wout_t.astype(bf),
        "b_row": b_row.astype(bf),
        "waug": waug,
        "ttok": ttok,
    }


def make_in_maps(inputs, cfg):
    shared, tgt_next = _prep_shared(inputs, cfg)
    return [dict(shared, **_prep_core(inputs, cfg, c, tgt_next))
            for c in range(NCORES)]


def combine(results, cfg):
    """results: list per core of {'s_out': [128, dec_mt], 't_out': [B, dec_steps]}"""
    dec_steps, dec_mt = cfg["dec_steps"], cfg["dec_mt"]
    S = np.zeros((128, dec_mt), np.float64)
    T = np.zeros((B, dec_steps), np.float64)
    for r in results:
        S += np.asarray(r["s_out"], np.float64)
        T += np.asarray(r["t_out"], np.float64)
    # S[p, mt] -> step = 2*mt + p//64, batch = p % 64
    s_tb = np.transpose(S.reshape(2, 64, dec_mt), (2, 0, 1)).reshape(-1, 64)
    s_tb = s_tb[:dec_steps]  # [dec_steps, B]
    t_tb = T.T  # [dec_steps, B]
    loss = np.sum(np.mean(np.log(s_tb) - t_tb, axis=1))
    return np.float32(loss)


@functools.lru_cache(maxsize=2)
def _get_compiled(key):
    cfg = _cfg()
    nc = build_program(cfg)
    return cfg, nc


def _run_hw(nc, in_maps):
    from concourse.bass_utils import run_bass_kernel_spmd
    res = run_bass_kernel_spmd(nc, in_maps, core_ids=list(range(NCORES)),
                               trace=False)
    return res.results


def kernel(**inputs):
    cfg, nc = _get_compiled("full")
    in_maps = make_in_maps(inputs, cfg)
    results = _run_hw(nc, in_maps)
    return combine(results, cfg)

